# revision 34
# baseline (speedup 1.0000x reference)
"""Distributed Trainium2 kernel for nn_Attention_73675868995842.

Sharding: 8-way head tensor-parallel (2 q heads + 1 kv head per core); hidden
states replicated (host-staged, pre-transposed + tiled); each core computes a
full [S, HID] partial of the output projection; host sums the 8 partials.

Per-core pipeline (raw Bass, hand-scheduled semaphores):
  P: qkv = hsT.T @ wqkv (fp32r)   -> rmsnorm(q,k) -> rope -> PE-transpose
     q,k to [d, s] layout; v cast to bf16.
  A: scores_b = qT.T @ kT (fp32r, per 512-col block) -> per-block max ->
     exp((s - m_b)*scale) w/ fused row-sum -> block fixup+normalize (bf16)
     -> DMA-transpose probs -> attnT += v.T @ probsT (bf16).
  O: out_partial = attnT.T @ wo (bf16) -> bf16 partial to DRAM.
"""
import contextlib
import numpy as np
import ml_dtypes

import concourse.bass as bass
from concourse import mybir
from concourse import bass_utils

F32 = mybir.dt.float32
F32R = mybir.dt.float32r
BF16 = mybir.dt.bfloat16
AX = mybir.AxisListType.X
Exp = mybir.ActivationFunctionType.Exp
Sqrt = mybir.ActivationFunctionType.Sqrt
Square = mybir.ActivationFunctionType.Square

B, S, HID = 1, 2048, 2048
H, KVH, HD = 16, 8, 128
NCORES = 8
HQ = H // NCORES            # 2 q heads per core
EPS = 1e-6
SCALE = float(np.float64(128.0) ** 0.5)   # reference divides by HD**-0.5
NT = S // 128               # 16 s-tiles
NB = S // 512               # 4 k-blocks per row
NI = HQ * NT                # 32 (h, qt) iterations
NJ = HQ * (S // 512)        # 8 AV groups

TRACE = False               # test.py flips this for timing runs
TRACE_DIR = None

_nc_cache = []


def build():
    nc = bass.Bass()
    hsT = nc.declare_dram_parameter("hsT", [NT, 128, 16, 128], F32R, isOutput=False)
    wqkv = nc.declare_dram_parameter("wqkv", [128, 16, 512], F32R, isOutput=False)
    wo = nc.declare_dram_parameter("wo", [128, HQ, 2048], BF16, isOutput=False)
    cosp = nc.declare_dram_parameter("cosp", [128, 16, 128], F32, isOutput=False)
    sinp = nc.declare_dram_parameter("sinp", [128, 16, 128], F32, isOutput=False)
    identp = nc.declare_dram_parameter("identp", [128, 128], F32, isOutput=False)
    identbp = nc.declare_dram_parameter("identbp", [128, 128], BF16, isOutput=False)
    constp = nc.declare_dram_parameter("constp", [128, 1], F32, isOutput=False)  # eps
    out = nc.declare_dram_parameter("out", [S, HID], BF16, isOutput=True)

    es = contextlib.ExitStack()

    def sb(name, shape, dt):
        return es.enter_context(nc.sbuf_tensor(name, shape, dt))

    def psum(name, shape, dt):
        return es.enter_context(nc.psum_tensor(name, shape, dt))

    def sem(name):
        return es.enter_context(nc.semaphore(name))

    with es:
        # ---- SBUF ----
        hs_sb = [sb(f"hs{p}", [128, 16, 128], F32R) for p in range(4)]
        wqkv_sb = sb("wqkv_sb", [128, 16, 512], F32R)
        wo_sb = sb("wo_sb", [128, HQ, 2048], BF16)
        cos_sb = sb("cos_sb", [128, 16, 128], F32)
        sin_sb = sb("sin_sb", [128, 16, 128], F32)
        ident = sb("ident", [128, 128], F32)
        identb = sb("identb", [128, 128], BF16)
        eps_t = sb("eps_t", [128, 1], F32)
        ssq = [sb(f"ssq{p}", [128, 3], F32) for p in range(2)]
        std = [sb(f"std{p}", [128, 3], F32) for p in range(2)]
        rstd = [sb(f"rstd{p}", [128, 3], F32) for p in range(2)]
        sq_scr = sb("sq_scr", [128, 3, 128], F32)
        qn = sb("qn", [128, 384], F32)
        tmp1 = sb("tmp1", [128, 384], F32)
        tmp2 = sb("tmp2", [128, 384], F32)
        qrope = [sb(f"qrope{p}", [128, 384], F32) for p in range(3)]
        qT = sb("qT", [128, HQ, S], F32R)
        kT = sb("kT", [128, S], F32R)
        v_sb = sb("v_sb", [128, 16, 128], BF16)
        attnT = sb("attnT", [128, HQ, S], BF16)
        negbm = [sb(f"negbm{p}", [128, 5], F32) for p in range(2)]
        biases = [sb(f"biases{p}", [128, 5], F32) for p in range(2)]
        sums = [sb(f"sums{p}", [128, 4], F32) for p in range(2)]
        cs = [sb(f"cs{p}", [128, 4], F32) for p in range(2)]
        t4 = [sb(f"t4{p}", [128, 4], F32) for p in range(2)]
        tot = [sb(f"tot{p}", [128, 1], F32) for p in range(2)]
        rcp = [sb(f"rcp{p}", [128, 1], F32) for p in range(2)]
        fct = [sb(f"fct{p}", [128, 4], F32) for p in range(2)]
        probs = [sb(f"probs{p}", [128, 2048], BF16) for p in range(3)]
        probsT = [sb(f"probsT{p}", [128, 16, 512], BF16) for p in range(2)]
        out_sb = [sb(f"out_sb{p}", [128, 2048], BF16) for p in range(2)]

        # ---- PSUM (8 banks total) ----
        ps_qkv = [psum(f"ps_qkv{p}", [128, 512], F32) for p in range(2)]  # P + O
        ps_tr = psum("ps_tr", [128, 512], F32)                            # P
        sc = psum("sc", [128, 2048], F32)                                 # A (4 banks)
        ps_av = psum("ps_av", [128, 512], F32)                            # A

        # ---- semaphores ----
        s_hin = [sem("s_hin0"), sem("s_hin1"), sem("s_hin2"), sem("s_hin3")]
        s_const = sem("s_const"); s_wq = sem("s_wq")
        s_qkv = sem("s_qkv");   s_sq = sem("s_sq");     s_std = sem("s_std")
        s_rstd = sem("s_rstd"); s_qn = sem("s_qn");     s_vcp = sem("s_vcp")
        s_ropem = sem("s_ropem"); s_roped = sem("s_roped")
        s_trmm = sem("s_trmm"); s_trcp = sem("s_trcp")
        s_scmm = sem("s_scmm"); s_bmax = sem("s_bmax"); s_gmin = sem("s_gmin")
        s_bias = sem("s_bias"); s_exp = sem("s_exp");   s_cs = sem("s_cs")
        s_t4 = sem("s_t4");     s_tot = sem("s_tot");   s_rcp = sem("s_rcp")
        s_fct = sem("s_fct");   s_norm = sem("s_norm")
        s_ptmm = sem("s_ptmm")
        s_ptcp0 = sem("s_ptcp0"); s_ptcp1 = sem("s_ptcp1")
        s_av = sem("s_av");   s_avcp = sem("s_avcp")
        s_omm = sem("s_omm");   s_ocp = sem("s_ocp");   s_ocp2 = sem("s_ocp2")
        s_dbg = sem("s_dbg")
        s_outd = [sem("s_outd0"), sem("s_outd1")]

        block = es.enter_context(nc.Block())

        def _av_group(te, j):
            te.wait_ge(s_ptcp0, 4 * (j + 1))
            te.wait_ge(s_ptcp1, 4 * (j + 1))
            if j == 0:
                te.wait_ge(s_vcp, NT)
            if j >= 1:
                te.wait_ge(s_avcp, j)                  # ps_av free
            for kc in range(16):
                te.matmul(ps_av[:], v_sb[:, kc, :], probsT[j % 2][:, kc, :],
                          start=(kc == 0), stop=(kc == 15)).then_maybe_inc(
                              (s_av, 1) if kc == 15 else None)

        def _pt_copy0(ve, pi):
            j = pi // 4
            ve.wait_ge(s_ptmm, 2 * pi + 1)
            if pi % 4 == 0 and j >= 2:
                ve.wait_ge(s_av, j - 1)                # probsT buf consumed
            ve.tensor_copy(out=probsT[j % 2][:, 0:8, (pi % 4) * 128:(pi % 4) * 128 + 128],
                           in_=ps_qkv[0].bitcast(BF16).rearrange(
                               "p (c d) -> p c d", c=8)).then_inc(s_ptcp0, 1)

        def _pt_copy1(se, pi):
            j = pi // 4
            se.wait_ge(s_ptmm, 2 * pi + 2)
            if pi % 4 == 0 and j >= 2:
                se.wait_ge(s_av, j - 1)                # probsT buf consumed
            se.copy(out=probsT[j % 2][:, 8:16, (pi % 4) * 128:(pi % 4) * 128 + 128],
                    in_=ps_qkv[1].bitcast(BF16).rearrange(
                        "p (c d) -> p c d", c=8)).then_inc(s_ptcp1, 1)

        def _tr_copy(se, st):
            se.wait_ge(s_trmm, st + 1)
            se.copy(out=qT[:, :, st * 128:(st + 1) * 128],
                    in_=ps_tr[:, 0:256].rearrange("p (h d) -> p h d", h=2))
            se.copy(out=kT[:, st * 128:(st + 1) * 128],
                    in_=ps_tr[:, 256:384]).then_inc(s_trcp, 1)

        def _tail_sec(ve, m):
            ve.wait_ge(s_exp, 4 * (m + 1))
            ve.reduce_sum(out=tot[m % 2][:], in_=sums[m % 2][:], axis=AX).then_inc(s_tot, 1)
            ve.wait_ge(s_tot, m + 1)
            ve.reciprocal(out=rcp[m % 2][:], in_=tot[m % 2][:]).then_inc(s_rcp, 1)
            ve.wait_ge(s_rcp, m + 1)
            for b in range(NB):
                ve.tensor_scalar_mul(out=probs[m % 3][:, b * 512:(b + 1) * 512],
                                     in0=probs[m % 3][:, b * 512:(b + 1) * 512],
                                     scalar1=rcp[m % 2][:]).then_maybe_inc(
                                         (s_norm, 1) if b == 3 else None)

        def _attn_copy(ve, j):
            h = j // 4
            qc = (j % 4) * 512
            ve.wait_ge(s_av, j + 1)
            ve.tensor_copy(out=attnT[:, h, qc:qc + 512], in_=ps_av[:]).then_inc(s_avcp, 1)

        # ---------------- SYNC: all DMA ----------------
        @block.sync
        def _(sy):
            sy.dma_start(out=hs_sb[0][:], in_=hsT[0]).then_inc(s_hin[0], 16)
            sy.dma_start(out=wqkv_sb[:], in_=wqkv[:]).then_inc(s_wq, 16)
            sy.dma_start(out=hs_sb[1][:], in_=hsT[1]).then_inc(s_hin[1], 16)
            sy.dma_start(out=ident[:], in_=identp[:]).then_inc(s_wq, 16)
            sy.dma_start(out=eps_t[:], in_=constp[:]).then_inc(s_const, 16)
            sy.dma_start(out=cos_sb[:], in_=cosp[:]).then_inc(s_const, 16)
            sy.dma_start(out=sin_sb[:], in_=sinp[:]).then_inc(s_const, 16)
            sy.dma_start(out=wo_sb[:], in_=wo[:]).then_inc(s_const, 16)
            sy.dma_start(out=identb[:], in_=identbp[:]).then_inc(s_const, 16)
            for st in range(2, NT):
                if st >= 4:
                    sy.wait_ge(s_qkv, st - 3)          # hs buf consumed
                sy.dma_start(out=hs_sb[st % 4][:], in_=hsT[st]).then_inc(s_hin[st % 4], 16)
            # phase O: partial out stores
            for st in range(NT):
                sy.wait_ge(s_ocp, 2 * (st + 1))
                sy.wait_ge(s_ocp2, 2 * (st + 1))
                sy.dma_start(out=out[st * 128:(st + 1) * 128, :],
                             in_=out_sb[st % 2][:]).then_inc(s_outd[st % 2], 16)

        # ---------------- TENSOR ----------------
        @block.tensor
        def _(te):
            te.wait_ge(s_wq, 32)
            # ---- phase P ----
            def _tr_group(pst):
                te.wait_ge(s_roped, pst + 1)
                if pst >= 1:
                    te.wait_ge(s_trcp, pst)            # ps_tr bank: copies of pst-1 done
                for idx in range(3):
                    te.matmul(ps_tr[:, idx * 128:(idx + 1) * 128],
                              qrope[pst % 3][:, idx * 128:(idx + 1) * 128],
                              ident[:], is_transpose=True,
                              start=True, stop=True).then_maybe_inc(
                                  (s_trmm, 1) if idx == 2 else None)

            qkv_banks = [ps_qkv[0][:], ps_qkv[1][:], sc[:, 0:512], sc[:, 512:1024]]
            for st in range(NT):
                te.wait_ge(s_hin[st % 4], 16 * (st // 4 + 1))
                if st >= 4:
                    te.wait_ge(s_qn, st - 3)           # psum buf: qn scale read done
                    te.wait_ge(s_vcp, st - 3)          # psum buf: v copy done
                pq = qkv_banks[st % 4]
                for c in range(16):
                    te.matmul(pq[:], hs_sb[st % 4][:, c, :], wqkv_sb[:, c, :],
                              start=(c == 0), stop=(c == 15)).then_maybe_inc(
                                  (s_qkv, 1) if c == 15 else None)
                if st >= 2:
                    _tr_group(st - 2)
            _tr_group(NT - 2)
            _tr_group(NT - 1)

            # ---- phase A ----
            te.wait_ge(s_trcp, NT)                     # qT/kT complete
            te.wait_ge(s_vcp, NT)                      # sc banks: last P readers done
            def _pt_rounds(te, pi):
                # PE-transpose probs[pi] (16 kc chunks) into ps_qkv banks (bf16 view)
                te.wait_ge(s_norm, pi + 1)
                for r in range(2):
                    if pi >= 1:
                        te.wait_ge((s_ptcp0, s_ptcp1)[r], pi)   # bank free
                    bank = ps_qkv[r].bitcast(BF16)              # [128, 1024]
                    for u in range(8):
                        kc = 8 * r + u
                        te.matmul(bank[:, u * 128:(u + 1) * 128],
                                  probs[pi % 3][:, kc * 128:(kc + 1) * 128],
                                  identb[:], is_transpose=True,
                                  start=True, stop=True).then_maybe_inc(
                                      (s_ptmm, 1) if u == 7 else None)

            for i in range(NI):
                h, qt = divmod(i, 16)
                for b in range(NB):
                    if i >= 1:
                        te.wait_ge(s_exp, 4 * (i - 1) + b + 1)  # sc bank b free
                    te.matmul(sc[:, b * 512:(b + 1) * 512],
                              qT[:, h, qt * 128:(qt + 1) * 128],
                              kT[:, b * 512:(b + 1) * 512],
                              start=True, stop=True).then_inc(s_scmm, 1)
                if i >= 2:
                    _pt_rounds(te, i - 2)
                # AV group j = i//4 - 1 interleaved one group behind
                if i % 4 == 3 and i >= 7:
                    _av_group(te, i // 4 - 1)
            _pt_rounds(te, NI - 2)
            _pt_rounds(te, NI - 1)
            _av_group(te, NJ - 1)

            # ---- phase O (cont.) ----
            te.wait_ge(s_avcp, NJ)
            o_banks = [ps_qkv[0][:], ps_qkv[1][:], sc[:, 0:512], sc[:, 512:1024]]
            for st in range(NT):
                for eb in range(4):
                    idx = st * 4 + eb
                    if idx >= 4:
                        pidx = idx - 4
                        if pidx % 2 == 0:
                            te.wait_ge(s_ocp, pidx // 2 + 1)
                        else:
                            te.wait_ge(s_ocp2, pidx // 2 + 1)
                    po = o_banks[idx % 4]
                    for h in range(HQ):
                        te.matmul(po[:], attnT[:, h, st * 128:(st + 1) * 128],
                                  wo_sb[:, h, eb * 512:(eb + 1) * 512],
                                  start=(h == 0), stop=(h == HQ - 1)).then_maybe_inc(
                                      (s_omm, 1) if h == HQ - 1 else None)

        # ---------------- SCALAR (ACT) ----------------
        @block.scalar
        def _(se):
            se.wait_ge(s_const, 80)
            # ---- phase P ----
            for st in range(NT):
                se.wait_ge(s_qkv, st + 1)
                if st >= 2:
                    se.wait_ge(s_rstd, st - 1)         # std buf: recip read done
                pq = [ps_qkv[0][:], ps_qkv[1][:], sc[:, 0:512], sc[:, 512:1024]][st % 4]
                for hh in range(3):
                    se.activation(out=sq_scr[:, hh, :], in_=pq[:, hh * 128:(hh + 1) * 128],
                                  func=Square,
                                  accum_out=ssq[st % 2][:, hh:hh + 1]).then_maybe_inc(
                                      (s_sq, 1) if hh == 2 else None)
                se.wait_ge(s_sq, st + 1)               # own squares done
                se.activation(out=std[st % 2][:], in_=ssq[st % 2][:], func=Sqrt,
                              scale=1.0 / 128.0, bias=eps_t[:]).then_inc(s_std, 1)
                # v copy (frees psum buf together with s_qn)
                se.copy(out=v_sb[:, st, :], in_=pq[:, 384:512]).then_inc(s_vcp, 1)
                if st >= 2:
                    _tr_copy(se, st - 2)

            _tr_copy(se, NT - 2)
            _tr_copy(se, NT - 1)

            # ---- phase A ----
            for i in range(NI):
                if i >= 2:
                    _pt_copy1(se, i - 2)
                se.wait_ge(s_gmin, i + 1)
                if i >= 3:
                    se.wait_ge(s_ptmm, 2 * (i - 2))    # probs buf free (transposes of i-3)
                if i >= 2:
                    se.wait_ge(s_tot, i - 1)           # sums buf free
                for b in range(NB):
                    se.activation(out=probs[i % 3][:, b * 512:(b + 1) * 512],
                                  in_=sc[:, b * 512:(b + 1) * 512], func=Exp,
                                  scale=1.0, bias=negbm[i % 2][:, 4:5],
                                  accum_out=sums[i % 2][:, b:b + 1]).then_inc(s_exp, 1)


            _pt_copy1(se, NI - 2)
            _pt_copy1(se, NI - 1)

            # ---- phase O ----
            for st in range(NT):
                for eb in range(4):
                    idx = st * 4 + eb
                    if eb % 2 != 0:
                        continue
                    se.wait_ge(s_omm, idx + 1)
                    if eb == 0 and st >= 2:
                        se.wait_ge(s_outd[st % 2], 16 * (st // 2))  # out_sb buf free
                    se.copy(out=out_sb[st % 2][:, eb * 512:(eb + 1) * 512],
                            in_=[ps_qkv[0][:], ps_qkv[1][:], sc[:, 0:512],
                                 sc[:, 512:1024]][idx % 4]).then_inc(s_ocp, 1)

        # ---------------- VECTOR (DVE) ----------------
        @block.vector
        def _(ve):
            ve.wait_ge(s_const, 80)                    # cos/sin loaded
            # ---- phase P ----
            for st in range(NT):
                ve.wait_ge(s_std, st + 1)
                ve.reciprocal(out=rstd[st % 2][:], in_=std[st % 2][:]).then_inc(s_rstd, 1)
                ve.wait_ge(s_rstd, st + 1)             # self RAW
                pq = [ps_qkv[0][:], ps_qkv[1][:], sc[:, 0:512], sc[:, 512:1024]][st % 4]
                for hh in range(3):
                    if hh < 2:   # q heads: also fold in the softmax scale sqrt(HD)
                        ve.tensor_scalar(out=qn[:, hh * 128:(hh + 1) * 128],
                                         in0=pq[:, hh * 128:(hh + 1) * 128],
                                         scalar1=rstd[st % 2][:, hh:hh + 1],
                                         scalar2=SCALE,
                                         op0=mybir.AluOpType.mult,
                                         op1=mybir.AluOpType.mult)
                    else:
                        ve.tensor_scalar_mul(out=qn[:, hh * 128:(hh + 1) * 128],
                                             in0=pq[:, hh * 128:(hh + 1) * 128],
                                             scalar1=rstd[st % 2][:, hh:hh + 1]).then_inc(s_qn, 1)
                ve.wait_ge(s_qn, st + 1)               # self RAW on qn
                if st >= 3:
                    ve.wait_ge(s_trmm, st - 2)         # qrope buf consumed by PE
                ct = cos_sb[:, st, :]
                stt = sin_sb[:, st, :]
                for hh in range(3):
                    c0 = hh * 128
                    ve.tensor_mul(out=tmp1[:, c0:c0 + 128], in0=qn[:, c0:c0 + 128], in1=ct)
                for hh in range(3):
                    c0 = hh * 128
                    ve.tensor_mul(out=tmp2[:, c0:c0 + 64],
                                  in0=qn[:, c0 + 64:c0 + 128], in1=stt[:, 0:64])
                    ve.tensor_mul(out=tmp2[:, c0 + 64:c0 + 128],
                                  in0=qn[:, c0:c0 + 64],
                                  in1=stt[:, 64:128]).then_maybe_inc(
                                      (s_ropem, 1) if hh == 2 else None)
                ve.wait_ge(s_ropem, st + 1)            # self RAW on tmp1/tmp2
                qr = qrope[st % 3]
                for hh in range(3):
                    c0 = hh * 128
                    ve.tensor_sub(out=qr[:, c0:c0 + 64],
                                  in0=tmp1[:, c0:c0 + 64], in1=tmp2[:, c0:c0 + 64])
                    ve.tensor_add(out=qr[:, c0 + 64:c0 + 128],
                                  in0=tmp1[:, c0 + 64:c0 + 128],
                                  in1=tmp2[:, c0 + 64:c0 + 128]).then_maybe_inc(
                                      (s_roped, 1) if hh == 2 else None)


            # ---- phase A ----
            for i in range(NI):
                if i >= 2:
                    ve.wait_ge(s_exp, 4 * (i - 1))     # negbm/biases consumed by ACT exp
                for b in range(NB):
                    ve.wait_ge(s_scmm, 4 * i + b + 1)
                    ve.reduce_max(out=negbm[i % 2][:, b:b + 1],
                                  in_=sc[:, b * 512:(b + 1) * 512], axis=AX,
                                  negate=True).then_maybe_inc(
                                      (s_bmax, 1) if b == 3 else None)
                ve.wait_ge(s_bmax, i + 1)              # self RAW
                ve.tensor_reduce(out=negbm[i % 2][:, 4:5], in_=negbm[i % 2][:, 0:4],
                                 axis=AX, op=mybir.AluOpType.min).then_inc(s_gmin, 1)

                # fill the exp(i) window with non-sc-bank PSUM copies
                if i >= 2:
                    _pt_copy0(ve, i - 2)
                if i % 4 == 3 and i >= 7:
                    _attn_copy(ve, i // 4 - 1)

                # tail for the PREVIOUS iteration (exp(i-1) already complete)
                if i >= 1:
                    _tail_sec(ve, i - 1)
            _tail_sec(ve, NI - 1)
            _pt_copy0(ve, NI - 2)
            _pt_copy0(ve, NI - 1)
            _attn_copy(ve, NJ - 1)

            # ---- phase O: odd-eb copies on DVE ----
            for st in range(NT):
                for eb in range(4):
                    idx = st * 4 + eb
                    if eb % 2 != 1:
                        continue
                    ve.wait_ge(s_omm, idx + 1)
                    if eb == 1 and st >= 2:
                        ve.wait_ge(s_outd[st % 2], 16 * (st // 2))  # out_sb buf free
                    ve.tensor_copy(out=out_sb[st % 2][:, eb * 512:(eb + 1) * 512],
                                   in_=[ps_qkv[0][:], ps_qkv[1][:], sc[:, 0:512],
                                        sc[:, 512:1024]][idx % 4]).then_inc(s_ocp2, 1)

    return nc


def _host_prep(hidden_states, cos, sin, wq, wk, wv, wo):
    hs = np.ascontiguousarray(hidden_states.reshape(S, HID), dtype=np.float32)
    # hsT tiles: t[st, p, c, s] = hs[st*128+s, c*128+p]
    hsT = np.ascontiguousarray(
        hs.reshape(NT, 128, 16, 128).transpose(0, 3, 2, 1))
    cos_t = np.ascontiguousarray(
        cos.reshape(NT, 128, HD).transpose(1, 0, 2), dtype=np.float32)
    sin_t = np.ascontiguousarray(
        sin.reshape(NT, 128, HD).transpose(1, 0, 2), dtype=np.float32)
    ident = np.eye(128, dtype=np.float32)
    identb = np.eye(128, dtype=np.float32).astype(ml_dtypes.bfloat16)
    eps_c = np.full((128, 1), EPS, dtype=np.float32)

    in_maps = []
    for g in range(NCORES):
        wq_g = wq[:, g * HQ * HD:(g + 1) * HQ * HD]          # [2048, 256]
        wk_g = wk[:, g * HD:(g + 1) * HD]                    # [2048, 128]
        wv_g = wv[:, g * HD:(g + 1) * HD]                    # [2048, 128]
        wqkv_g = np.concatenate([wq_g, wk_g, wv_g], axis=1)  # [2048, 512]
        wqkv_t = np.ascontiguousarray(
            wqkv_g.reshape(16, 128, 512).transpose(1, 0, 2), dtype=np.float32)
        wo_g = wo[g * HQ * HD:(g + 1) * HQ * HD, :]          # [256, 2048]
        wo_t = np.ascontiguousarray(
            wo_g.reshape(HQ, 128, HID).transpose(1, 0, 2)).astype(ml_dtypes.bfloat16)
        in_maps.append({
            "hsT": hsT, "wqkv": wqkv_t, "wo": wo_t,
            "cosp": cos_t, "sinp": sin_t, "identp": ident, "identbp": identb,
            "constp": eps_c,
        })
    return in_maps


def kernel(hidden_states, cos, sin, wq, wk, wv, wo):
    hidden_states = np.asarray(hidden_states, dtype=np.float32)
    cos = np.asarray(cos, dtype=np.float32).reshape(S, HD)
    sin = np.asarray(sin, dtype=np.float32).reshape(S, HD)
    wq = np.asarray(wq, dtype=np.float32)
    wk = np.asarray(wk, dtype=np.float32)
    wv = np.asarray(wv, dtype=np.float32)
    wo = np.asarray(wo, dtype=np.float32)

    in_maps = _host_prep(hidden_states, cos, sin, wq, wk, wv, wo)
    if not _nc_cache:
        _nc_cache.append(build())
    nc = _nc_cache[0]
    kw = {}
    if TRACE:
        import tempfile
        kw = dict(trace=True, tmpdir=tempfile.mkdtemp(prefix="attn_trace_"))
    res = bass_utils.run_bass_kernel_spmd(nc, in_maps, list(range(NCORES)), **kw)
    if TRACE:
        print("HW exec time: %d ns" % res.exec_time_ns)
    acc = np.zeros((S, HID), dtype=np.float32)
    for g in range(NCORES):
        acc += res.results[g]["out"].astype(np.float32)
    return acc.reshape(B, S, HID)


# revision 35
# speedup vs baseline: 1.0349x; 1.0349x over previous
"""Distributed Trainium2 kernel for nn_Attention_73675868995842.

Sharding: 8-way head tensor-parallel (2 q heads + 1 kv head per core); hidden
states replicated (host-staged, pre-transposed + tiled); each core computes a
full [S, HID] partial of the output projection; host sums the 8 partials.

Per-core pipeline (raw Bass, hand-scheduled semaphores):
  P: qkv = hsT.T @ wqkv (fp32r)   -> rmsnorm(q,k) -> rope -> PE-transpose
     q,k to [d, s] layout; v cast to bf16.
  A: scores_b = qT.T @ kT (fp32r, per 512-col block) -> per-block max ->
     exp((s - m_b)*scale) w/ fused row-sum -> block fixup+normalize (bf16)
     -> DMA-transpose probs -> attnT += v.T @ probsT (bf16).
  O: out_partial = attnT.T @ wo (bf16) -> bf16 partial to DRAM.
"""
import contextlib
import numpy as np
import ml_dtypes

import concourse.bass as bass
from concourse import mybir
from concourse import bass_utils

F32 = mybir.dt.float32
F32R = mybir.dt.float32r
BF16 = mybir.dt.bfloat16
AX = mybir.AxisListType.X
Exp = mybir.ActivationFunctionType.Exp
Sqrt = mybir.ActivationFunctionType.Sqrt
Square = mybir.ActivationFunctionType.Square

B, S, HID = 1, 2048, 2048
H, KVH, HD = 16, 8, 128
NCORES = 8
HQ = H // NCORES            # 2 q heads per core
EPS = 1e-6
SCALE = float(np.float64(128.0) ** 0.5)   # reference divides by HD**-0.5
NT = S // 128               # 16 s-tiles
NB = S // 512               # 4 k-blocks per row
NI = HQ * NT                # 32 (h, qt) iterations
NJ = HQ * (S // 512)        # 8 AV groups

TRACE = False               # test.py flips this for timing runs
TRACE_DIR = None

_nc_cache = []


def build():
    nc = bass.Bass()
    hsT = nc.declare_dram_parameter("hsT", [NT, 128, 16, 128], F32R, isOutput=False)
    wqkv = nc.declare_dram_parameter("wqkv", [128, 16, 512], F32R, isOutput=False)
    wo = nc.declare_dram_parameter("wo", [128, HQ, 2048], BF16, isOutput=False)
    cosp = nc.declare_dram_parameter("cosp", [128, 16, 128], F32, isOutput=False)
    sinp = nc.declare_dram_parameter("sinp", [128, 16, 128], F32, isOutput=False)
    identp = nc.declare_dram_parameter("identp", [128, 128], F32, isOutput=False)
    identbp = nc.declare_dram_parameter("identbp", [128, 128], BF16, isOutput=False)
    constp = nc.declare_dram_parameter("constp", [128, 1], F32, isOutput=False)  # eps
    out = nc.declare_dram_parameter("out", [S, HID], BF16, isOutput=True)

    es = contextlib.ExitStack()

    def sb(name, shape, dt):
        return es.enter_context(nc.sbuf_tensor(name, shape, dt))

    def psum(name, shape, dt):
        return es.enter_context(nc.psum_tensor(name, shape, dt))

    def sem(name):
        return es.enter_context(nc.semaphore(name))

    with es:
        # ---- SBUF ----
        hs_sb = [sb(f"hs{p}", [128, 16, 128], F32R) for p in range(4)]
        wqkv_sb = sb("wqkv_sb", [128, 16, 512], F32R)
        wo_sb = sb("wo_sb", [128, HQ, 2048], BF16)
        cos_sb = sb("cos_sb", [128, 16, 128], F32)
        sin_sb = sb("sin_sb", [128, 16, 128], F32)
        ident = sb("ident", [128, 128], F32)
        identb = sb("identb", [128, 128], BF16)
        eps_t = sb("eps_t", [128, 1], F32)
        ssq = [sb(f"ssq{p}", [128, 3], F32) for p in range(2)]
        std = [sb(f"std{p}", [128, 3], F32) for p in range(2)]
        rstd = [sb(f"rstd{p}", [128, 3], F32) for p in range(2)]
        sq_scr = sb("sq_scr", [128, 3, 128], F32)
        qn = sb("qn", [128, 384], F32)
        tmp1 = sb("tmp1", [128, 384], F32)
        tmp2 = sb("tmp2", [128, 384], F32)
        qrope = [sb(f"qrope{p}", [128, 384], F32) for p in range(3)]
        qT = sb("qT", [128, HQ, S], F32R)
        kT = sb("kT", [128, S], F32R)
        v_sb = sb("v_sb", [128, 16, 128], BF16)
        attnT = sb("attnT", [128, HQ, S], BF16)
        negbm = [sb(f"negbm{p}", [128, 5], F32) for p in range(2)]
        biases = [sb(f"biases{p}", [128, 5], F32) for p in range(2)]
        sums = [sb(f"sums{p}", [128, 4], F32) for p in range(2)]
        cs = [sb(f"cs{p}", [128, 4], F32) for p in range(2)]
        t4 = [sb(f"t4{p}", [128, 4], F32) for p in range(2)]
        tot = [sb(f"tot{p}", [128, 1], F32) for p in range(2)]
        rcp = [sb(f"rcp{p}", [128, 1], F32) for p in range(2)]
        fct = [sb(f"fct{p}", [128, 4], F32) for p in range(2)]
        probs = [sb(f"probs{p}", [128, 2048], BF16) for p in range(3)]
        probsT = [sb(f"probsT{p}", [128, 16, 512], BF16) for p in range(2)]
        out_sb = [sb(f"out_sb{p}", [128, 2048], BF16) for p in range(2)]

        # ---- PSUM (8 banks total) ----
        ps_qkv = [psum(f"ps_qkv{p}", [128, 512], F32) for p in range(2)]  # P + O
        ps_tr = psum("ps_tr", [128, 512], F32)                            # P
        sc = psum("sc", [128, 2048], F32)                                 # A (4 banks)
        ps_av = psum("ps_av", [128, 512], F32)                            # A

        # ---- semaphores ----
        s_hin = [sem("s_hin0"), sem("s_hin1"), sem("s_hin2"), sem("s_hin3")]
        s_const = sem("s_const"); s_wq = sem("s_wq")
        s_qkv = sem("s_qkv");   s_sq = sem("s_sq");     s_std = sem("s_std")
        s_rstd = sem("s_rstd"); s_qn = sem("s_qn");     s_vcp = sem("s_vcp")
        s_ropem = sem("s_ropem"); s_roped = sem("s_roped")
        s_trmm = sem("s_trmm"); s_trcp = sem("s_trcp")
        s_scmm = sem("s_scmm"); s_bmax = sem("s_bmax"); s_gmin = sem("s_gmin")
        s_bias = sem("s_bias"); s_exp = sem("s_exp");   s_cs = sem("s_cs")
        s_t4 = sem("s_t4");     s_tot = sem("s_tot");   s_rcp = sem("s_rcp")
        s_fct = sem("s_fct");   s_norm = sem("s_norm")
        s_ptmm = sem("s_ptmm")
        s_ptcp0 = sem("s_ptcp0"); s_ptcp1 = sem("s_ptcp1")
        s_av = sem("s_av");   s_avcp = sem("s_avcp")
        s_omm = sem("s_omm");   s_ocp = sem("s_ocp");   s_ocp2 = sem("s_ocp2")
        s_dbg = sem("s_dbg")
        s_outd = [sem("s_outd0"), sem("s_outd1")]

        block = es.enter_context(nc.Block())

        def _av_group(te, j):
            te.wait_ge(s_ptcp0, 4 * (j + 1))
            te.wait_ge(s_ptcp1, 4 * (j + 1))
            if j == 0:
                te.wait_ge(s_vcp, NT)
            if j >= 1:
                te.wait_ge(s_avcp, j)                  # ps_av free
            for kc in range(16):
                te.matmul(ps_av[:], v_sb[:, kc, :], probsT[j % 2][:, kc, :],
                          start=(kc == 0), stop=(kc == 15)).then_maybe_inc(
                              (s_av, 1) if kc == 15 else None)

        def _pt_copy0(ve, pi):
            j = pi // 4
            ve.wait_ge(s_ptmm, 2 * pi + 1)
            if pi % 4 == 0 and j >= 2:
                ve.wait_ge(s_av, j - 1)                # probsT buf consumed
            ve.tensor_copy(out=probsT[j % 2][:, 0:8, (pi % 4) * 128:(pi % 4) * 128 + 128],
                           in_=ps_qkv[0].bitcast(BF16).rearrange(
                               "p (c d) -> p c d", c=8)).then_inc(s_ptcp0, 1)

        def _pt_copy1(se, pi):
            j = pi // 4
            se.wait_ge(s_ptmm, 2 * pi + 2)
            if pi % 4 == 0 and j >= 2:
                se.wait_ge(s_av, j - 1)                # probsT buf consumed
            se.copy(out=probsT[j % 2][:, 8:16, (pi % 4) * 128:(pi % 4) * 128 + 128],
                    in_=ps_qkv[1].bitcast(BF16).rearrange(
                        "p (c d) -> p c d", c=8)).then_inc(s_ptcp1, 1)

        def _tr_copy(se, st):
            se.wait_ge(s_trmm, st + 1)
            se.copy(out=qT[:, :, st * 128:(st + 1) * 128],
                    in_=ps_tr[:, 0:256].rearrange("p (h d) -> p h d", h=2))
            se.copy(out=kT[:, st * 128:(st + 1) * 128],
                    in_=ps_tr[:, 256:384]).then_inc(s_trcp, 1)

        def _tail_sec(ve, m):
            ve.wait_ge(s_exp, 4 * (m + 1))
            ve.reduce_sum(out=tot[m % 2][:], in_=sums[m % 2][:], axis=AX).then_inc(s_tot, 1)
            ve.wait_ge(s_tot, m + 1)
            ve.reciprocal(out=rcp[m % 2][:], in_=tot[m % 2][:]).then_inc(s_rcp, 1)
            ve.wait_ge(s_rcp, m + 1)
            for b in range(NB):
                ve.tensor_scalar_mul(out=probs[m % 3][:, b * 512:(b + 1) * 512],
                                     in0=probs[m % 3][:, b * 512:(b + 1) * 512],
                                     scalar1=rcp[m % 2][:]).then_maybe_inc(
                                         (s_norm, 1) if b == 3 else None)

        def _attn_copy(ve, j):
            h = j // 4
            qc = (j % 4) * 512
            ve.wait_ge(s_av, j + 1)
            ve.tensor_copy(out=attnT[:, h, qc:qc + 512], in_=ps_av[:]).then_inc(s_avcp, 1)

        # ---------------- SYNC: all DMA ----------------
        @block.sync
        def _(sy):
            sy.dma_start(out=hs_sb[0][:], in_=hsT[0]).then_inc(s_hin[0], 16)
            sy.dma_start(out=wqkv_sb[:], in_=wqkv[:]).then_inc(s_wq, 16)
            sy.dma_start(out=hs_sb[1][:], in_=hsT[1]).then_inc(s_hin[1], 16)
            sy.dma_start(out=ident[:], in_=identp[:]).then_inc(s_wq, 16)
            sy.dma_start(out=eps_t[:], in_=constp[:]).then_inc(s_const, 16)
            sy.dma_start(out=cos_sb[:], in_=cosp[:]).then_inc(s_const, 16)
            sy.dma_start(out=sin_sb[:], in_=sinp[:]).then_inc(s_const, 16)
            sy.dma_start(out=wo_sb[:], in_=wo[:]).then_inc(s_const, 16)
            sy.dma_start(out=identb[:], in_=identbp[:]).then_inc(s_const, 16)
            for st in range(2, NT):
                if st >= 4:
                    sy.wait_ge(s_qkv, st - 3)          # hs buf consumed
                sy.dma_start(out=hs_sb[st % 4][:], in_=hsT[st]).then_inc(s_hin[st % 4], 16)
            # phase O: partial out stores
            for st in range(NT):
                sy.wait_ge(s_ocp, 2 * (st + 1))
                sy.wait_ge(s_ocp2, 2 * (st + 1))
                sy.dma_start(out=out[st * 128:(st + 1) * 128, :],
                             in_=out_sb[st % 2][:]).then_inc(s_outd[st % 2], 16)

        # ---------------- TENSOR ----------------
        @block.tensor
        def _(te):
            te.wait_ge(s_wq, 32)
            # ---- phase P ----
            def _tr_group(pst):
                te.wait_ge(s_roped, pst + 1)
                if pst >= 1:
                    te.wait_ge(s_trcp, pst)            # ps_tr bank: copies of pst-1 done
                for idx in range(3):
                    te.matmul(ps_tr[:, idx * 128:(idx + 1) * 128],
                              qrope[pst % 3][:, idx * 128:(idx + 1) * 128],
                              ident[:], is_transpose=True,
                              start=True, stop=True).then_maybe_inc(
                                  (s_trmm, 1) if idx == 2 else None)

            qkv_banks = [ps_qkv[0][:], ps_qkv[1][:], sc[:, 0:512], sc[:, 512:1024]]
            for st in range(NT):
                te.wait_ge(s_hin[st % 4], 16 * (st // 4 + 1))
                if st >= 4:
                    te.wait_ge(s_qn, st - 3)           # psum buf: qn scale read done
                    te.wait_ge(s_vcp, st - 3)          # psum buf: v copy done
                pq = qkv_banks[st % 4]
                for c in range(16):
                    te.matmul(pq[:], hs_sb[st % 4][:, c, :], wqkv_sb[:, c, :],
                              start=(c == 0), stop=(c == 15)).then_maybe_inc(
                                  (s_qkv, 1) if c == 15 else None)
                if st >= 2:
                    _tr_group(st - 2)
            _tr_group(NT - 2)
            _tr_group(NT - 1)

            # ---- phase A ----
            te.wait_ge(s_trcp, NT)                     # qT/kT complete
            te.wait_ge(s_vcp, NT)                      # sc banks: last P readers done
            def _pt_rounds(te, pi):
                # PE-transpose probs[pi] (16 kc chunks) into ps_qkv banks (bf16 view)
                te.wait_ge(s_norm, pi + 1)
                for r in range(2):
                    if pi >= 1:
                        te.wait_ge((s_ptcp0, s_ptcp1)[r], pi)   # bank free
                    bank = ps_qkv[r].bitcast(BF16)              # [128, 1024]
                    for u in range(8):
                        kc = 8 * r + u
                        te.matmul(bank[:, u * 128:(u + 1) * 128],
                                  probs[pi % 3][:, kc * 128:(kc + 1) * 128],
                                  identb[:], is_transpose=True,
                                  start=True, stop=True).then_maybe_inc(
                                      (s_ptmm, 1) if u == 7 else None)

            for i in range(NI):
                h, qt = divmod(i, 16)
                for b in range(NB):
                    if i >= 1:
                        te.wait_ge(s_exp, 4 * (i - 1) + b + 1)  # sc bank b free
                    te.matmul(sc[:, b * 512:(b + 1) * 512],
                              qT[:, h, qt * 128:(qt + 1) * 128],
                              kT[:, b * 512:(b + 1) * 512],
                              start=True, stop=True).then_inc(s_scmm, 1)
                if i >= 2:
                    _pt_rounds(te, i - 2)
                # AV group j = i//4 - 1 interleaved one group behind
                if i % 4 == 3 and i >= 7:
                    _av_group(te, i // 4 - 1)
            _pt_rounds(te, NI - 2)
            _pt_rounds(te, NI - 1)
            _av_group(te, NJ - 1)

            # ---- phase O (cont.) ----
            te.wait_ge(s_avcp, NJ)
            o_banks = [ps_qkv[0][:], ps_qkv[1][:], sc[:, 0:512], sc[:, 512:1024]]
            for st in range(NT):
                for eb in range(4):
                    idx = st * 4 + eb
                    if idx >= 4:
                        pidx = idx - 4
                        if pidx % 2 == 0:
                            te.wait_ge(s_ocp, pidx // 2 + 1)
                        else:
                            te.wait_ge(s_ocp2, pidx // 2 + 1)
                    po = o_banks[idx % 4]
                    for h in range(HQ):
                        te.matmul(po[:], attnT[:, h, st * 128:(st + 1) * 128],
                                  wo_sb[:, h, eb * 512:(eb + 1) * 512],
                                  start=(h == 0), stop=(h == HQ - 1)).then_maybe_inc(
                                      (s_omm, 1) if h == HQ - 1 else None)

        # ---------------- SCALAR (ACT) ----------------
        @block.scalar
        def _(se):
            se.wait_ge(s_const, 80)
            # ---- phase P ----
            for st in range(NT):
                se.wait_ge(s_qkv, st + 1)
                if st >= 2:
                    se.wait_ge(s_rstd, st - 1)         # std buf: recip read done
                pq = [ps_qkv[0][:], ps_qkv[1][:], sc[:, 0:512], sc[:, 512:1024]][st % 4]
                for hh in range(3):
                    se.activation(out=sq_scr[:, hh, :], in_=pq[:, hh * 128:(hh + 1) * 128],
                                  func=Square,
                                  accum_out=ssq[st % 2][:, hh:hh + 1]).then_maybe_inc(
                                      (s_sq, 1) if hh == 2 else None)
                se.wait_ge(s_sq, st + 1)               # own squares done
                se.activation(out=std[st % 2][:], in_=ssq[st % 2][:], func=Sqrt,
                              scale=1.0 / 128.0, bias=eps_t[:]).then_inc(s_std, 1)
                # v copy (frees psum buf together with s_qn)
                se.copy(out=v_sb[:, st, :], in_=pq[:, 384:512]).then_inc(s_vcp, 1)
                if st >= 2:
                    _tr_copy(se, st - 2)

            _tr_copy(se, NT - 2)
            _tr_copy(se, NT - 1)

            # ---- phase A ----
            for i in range(NI):
                if i >= 2:
                    _pt_copy1(se, i - 2)
                se.wait_ge(s_scmm, 4 * (i + 1))
                se.wait_ge(s_gmin, i + 1)
                if i >= 3:
                    se.wait_ge(s_ptmm, 2 * (i - 2))    # probs buf free (transposes of i-3)
                if i >= 2:
                    se.wait_ge(s_tot, i - 1)           # sums buf free
                for b in range(NB):
                    se.activation(out=probs[i % 3][:, b * 512:(b + 1) * 512],
                                  in_=sc[:, b * 512:(b + 1) * 512], func=Exp,
                                  scale=1.0, bias=negbm[i % 2][:, 4:5],
                                  accum_out=sums[i % 2][:, b:b + 1]).then_inc(s_exp, 1)


            _pt_copy1(se, NI - 2)
            _pt_copy1(se, NI - 1)

            # ---- phase O ----
            for st in range(NT):
                for eb in range(4):
                    idx = st * 4 + eb
                    if eb % 2 != 0:
                        continue
                    se.wait_ge(s_omm, idx + 1)
                    if eb == 0 and st >= 2:
                        se.wait_ge(s_outd[st % 2], 16 * (st // 2))  # out_sb buf free
                    se.copy(out=out_sb[st % 2][:, eb * 512:(eb + 1) * 512],
                            in_=[ps_qkv[0][:], ps_qkv[1][:], sc[:, 0:512],
                                 sc[:, 512:1024]][idx % 4]).then_inc(s_ocp, 1)

        # ---------------- VECTOR (DVE) ----------------
        @block.vector
        def _(ve):
            ve.wait_ge(s_const, 80)                    # cos/sin loaded
            # ---- phase P ----
            for st in range(NT):
                ve.wait_ge(s_std, st + 1)
                ve.reciprocal(out=rstd[st % 2][:], in_=std[st % 2][:]).then_inc(s_rstd, 1)
                ve.wait_ge(s_rstd, st + 1)             # self RAW
                pq = [ps_qkv[0][:], ps_qkv[1][:], sc[:, 0:512], sc[:, 512:1024]][st % 4]
                for hh in range(3):
                    if hh < 2:   # q heads: also fold in the softmax scale sqrt(HD)
                        ve.tensor_scalar(out=qn[:, hh * 128:(hh + 1) * 128],
                                         in0=pq[:, hh * 128:(hh + 1) * 128],
                                         scalar1=rstd[st % 2][:, hh:hh + 1],
                                         scalar2=SCALE,
                                         op0=mybir.AluOpType.mult,
                                         op1=mybir.AluOpType.mult)
                    else:
                        ve.tensor_scalar_mul(out=qn[:, hh * 128:(hh + 1) * 128],
                                             in0=pq[:, hh * 128:(hh + 1) * 128],
                                             scalar1=rstd[st % 2][:, hh:hh + 1]).then_inc(s_qn, 1)
                ve.wait_ge(s_qn, st + 1)               # self RAW on qn
                if st >= 3:
                    ve.wait_ge(s_trmm, st - 2)         # qrope buf consumed by PE
                ct = cos_sb[:, st, :]
                stt = sin_sb[:, st, :]
                for hh in range(3):
                    c0 = hh * 128
                    ve.tensor_mul(out=tmp1[:, c0:c0 + 128], in0=qn[:, c0:c0 + 128], in1=ct)
                for hh in range(3):
                    c0 = hh * 128
                    ve.tensor_mul(out=tmp2[:, c0:c0 + 64],
                                  in0=qn[:, c0 + 64:c0 + 128], in1=stt[:, 0:64])
                    ve.tensor_mul(out=tmp2[:, c0 + 64:c0 + 128],
                                  in0=qn[:, c0:c0 + 64],
                                  in1=stt[:, 64:128]).then_maybe_inc(
                                      (s_ropem, 1) if hh == 2 else None)
                ve.wait_ge(s_ropem, st + 1)            # self RAW on tmp1/tmp2
                qr = qrope[st % 3]
                for hh in range(3):
                    c0 = hh * 128
                    ve.tensor_sub(out=qr[:, c0:c0 + 64],
                                  in0=tmp1[:, c0:c0 + 64], in1=tmp2[:, c0:c0 + 64])
                    ve.tensor_add(out=qr[:, c0 + 64:c0 + 128],
                                  in0=tmp1[:, c0 + 64:c0 + 128],
                                  in1=tmp2[:, c0 + 64:c0 + 128]).then_maybe_inc(
                                      (s_roped, 1) if hh == 2 else None)


            # ---- phase A ----
            for i in range(NI):
                if i >= 2:
                    ve.wait_ge(s_exp, 4 * (i - 1))     # negbm/biases consumed by ACT exp
                for b in range(NB):
                    ve.wait_ge(s_scmm, 4 * i + b + 1)
                    ve.reduce_max(out=negbm[i % 2][:, b:b + 1],
                                  in_=sc[:, b * 512:(b + 1) * 512], axis=AX,
                                  negate=True).then_maybe_inc(
                                      (s_bmax, 1) if b == 3 else None)
                ve.wait_ge(s_bmax, i + 1)              # self RAW
                ve.tensor_reduce(out=negbm[i % 2][:, 4:5], in_=negbm[i % 2][:, 0:4],
                                 axis=AX, op=mybir.AluOpType.min).then_inc(s_gmin, 1)

                # fill the exp(i) window with non-sc-bank PSUM copies
                if i >= 2:
                    _pt_copy0(ve, i - 2)
                if i % 4 == 3 and i >= 7:
                    _attn_copy(ve, i // 4 - 1)

                # tail for the PREVIOUS iteration (exp(i-1) already complete)
                if i >= 1:
                    _tail_sec(ve, i - 1)
            _tail_sec(ve, NI - 1)
            _pt_copy0(ve, NI - 2)
            _pt_copy0(ve, NI - 1)
            _attn_copy(ve, NJ - 1)

            # ---- phase O: odd-eb copies on DVE ----
            for st in range(NT):
                for eb in range(4):
                    idx = st * 4 + eb
                    if eb % 2 != 1:
                        continue
                    ve.wait_ge(s_omm, idx + 1)
                    if eb == 1 and st >= 2:
                        ve.wait_ge(s_outd[st % 2], 16 * (st // 2))  # out_sb buf free
                    ve.tensor_copy(out=out_sb[st % 2][:, eb * 512:(eb + 1) * 512],
                                   in_=[ps_qkv[0][:], ps_qkv[1][:], sc[:, 0:512],
                                        sc[:, 512:1024]][idx % 4]).then_inc(s_ocp2, 1)

    return nc


def _host_prep(hidden_states, cos, sin, wq, wk, wv, wo):
    hs = np.ascontiguousarray(hidden_states.reshape(S, HID), dtype=np.float32)
    # hsT tiles: t[st, p, c, s] = hs[st*128+s, c*128+p]
    hsT = np.ascontiguousarray(
        hs.reshape(NT, 128, 16, 128).transpose(0, 3, 2, 1))
    cos_t = np.ascontiguousarray(
        cos.reshape(NT, 128, HD).transpose(1, 0, 2), dtype=np.float32)
    sin_t = np.ascontiguousarray(
        sin.reshape(NT, 128, HD).transpose(1, 0, 2), dtype=np.float32)
    ident = np.eye(128, dtype=np.float32)
    identb = np.eye(128, dtype=np.float32).astype(ml_dtypes.bfloat16)
    eps_c = np.full((128, 1), EPS, dtype=np.float32)

    in_maps = []
    for g in range(NCORES):
        wq_g = wq[:, g * HQ * HD:(g + 1) * HQ * HD]          # [2048, 256]
        wk_g = wk[:, g * HD:(g + 1) * HD]                    # [2048, 128]
        wv_g = wv[:, g * HD:(g + 1) * HD]                    # [2048, 128]
        wqkv_g = np.concatenate([wq_g, wk_g, wv_g], axis=1)  # [2048, 512]
        wqkv_t = np.ascontiguousarray(
            wqkv_g.reshape(16, 128, 512).transpose(1, 0, 2), dtype=np.float32)
        wo_g = wo[g * HQ * HD:(g + 1) * HQ * HD, :]          # [256, 2048]
        wo_t = np.ascontiguousarray(
            wo_g.reshape(HQ, 128, HID).transpose(1, 0, 2)).astype(ml_dtypes.bfloat16)
        in_maps.append({
            "hsT": hsT, "wqkv": wqkv_t, "wo": wo_t,
            "cosp": cos_t, "sinp": sin_t, "identp": ident, "identbp": identb,
            "constp": eps_c,
        })
    return in_maps


def kernel(hidden_states, cos, sin, wq, wk, wv, wo):
    hidden_states = np.asarray(hidden_states, dtype=np.float32)
    cos = np.asarray(cos, dtype=np.float32).reshape(S, HD)
    sin = np.asarray(sin, dtype=np.float32).reshape(S, HD)
    wq = np.asarray(wq, dtype=np.float32)
    wk = np.asarray(wk, dtype=np.float32)
    wv = np.asarray(wv, dtype=np.float32)
    wo = np.asarray(wo, dtype=np.float32)

    in_maps = _host_prep(hidden_states, cos, sin, wq, wk, wv, wo)
    if not _nc_cache:
        _nc_cache.append(build())
    nc = _nc_cache[0]
    kw = {}
    if TRACE:
        import tempfile
        kw = dict(trace=True, tmpdir=tempfile.mkdtemp(prefix="attn_trace_"))
    res = bass_utils.run_bass_kernel_spmd(nc, in_maps, list(range(NCORES)), **kw)
    if TRACE:
        print("HW exec time: %d ns" % res.exec_time_ns)
    acc = np.zeros((S, HID), dtype=np.float32)
    for g in range(NCORES):
        acc += res.results[g]["out"].astype(np.float32)
    return acc.reshape(B, S, HID)
